# revision 19
# baseline (speedup 1.0000x reference)
"""AWQLinear forward on 8 Trainium2 NeuronCores.

y = x @ dequant(W)^T + bias, where
  dequant(W)[o,k] = (wint[o,k] - zero[o,g(k)]) * scale[o,g(k)] / awq[k],
  g(k) = k // 128.

Sharding: tensor-parallel over out_features (11008 -> 8 x 1376). x is
replicated; each core dequantizes its weight slice on-device, runs the
matmul in bf16 with fp32 PSUM accumulation, and writes its output
column block. The host concatenates the column blocks.

Device-side layouts (all host-side reshapes, so the contraction dim
K=in_features lands on SBUF partitions and DMA rows are contiguous):
  xt     (T/TC, TC/128, 128, 32, 128) bf16  x pre-tiled:
         [chunk, t_tile, k_in_group, group, token]
  wT     (4096, 1376) bf16         weight_int slice transposed (int4, exact in bf16)
  scaleT (32, 1376)   f32          scale_per_group slice transposed
  zeroT  (32, 1376)   f32          zero_per_group slice transposed (omitted when 0)
  awq    (4096,)      f32
  bias   (1376,)      f32
"""

import os
import sys

for _p in ("/opt/trn_rl_repo", "/opt/pypackages"):
    if os.path.isdir(_p) and _p not in sys.path:
        sys.path.append(_p)

import numpy as np
import ml_dtypes

import concourse.bass as bass
import concourse.mybir as mybir
import concourse.tile as tile
from concourse import bacc
from concourse.bass_utils import run_bass_kernel_spmd

BF16 = ml_dtypes.bfloat16

T_FULL = 8192      # tokens
IN = 4096          # in_features (contraction)
OUT = 11008        # out_features
N_CORES = 8
OUT_S = OUT // N_CORES   # 1376 per core
GS = 128           # quant group size == SBUF partition count
G = IN // GS       # 32 groups
T_CHUNK = 512      # tokens per x DMA chunk

f32 = mybir.dt.float32
bf16 = mybir.dt.bfloat16


def chunk_for(use_zero, t_tokens=T_FULL):
    # the zero path carries extra dequant temporaries; shrink x chunks
    return min(256 if use_zero else T_CHUNK, t_tokens)


def build_nc(t_tokens=T_FULL, out_s=OUT_S, t_chunk=None, use_zero=True):
    """Build the per-core Bass program (same program on all 8 cores)."""
    if t_chunk is None:
        t_chunk = chunk_for(use_zero, t_tokens)
    t_chunk = min(t_chunk, t_tokens)
    assert t_tokens % t_chunk == 0 and t_chunk % 128 == 0
    n_ch = t_tokens // t_chunk
    n_tt = t_chunk // 128
    nc = bacc.Bacc("TRN2", target_bir_lowering=False, debug=False)

    xt = nc.dram_tensor("xt", [n_ch, n_tt, GS, G, 128], bf16,
                        kind="ExternalInput").ap()
    wT = nc.dram_tensor("wT", [IN, out_s], bf16, kind="ExternalInput").ap()
    scaleT = nc.dram_tensor("scaleT", [G, out_s], f32, kind="ExternalInput").ap()
    if use_zero:
        zeroT = nc.dram_tensor("zeroT", [G, out_s], f32,
                               kind="ExternalInput").ap()
    awq = nc.dram_tensor("awq", [IN], f32, kind="ExternalInput").ap()
    bias = nc.dram_tensor("bias", [out_s], f32, kind="ExternalInput").ap()
    y = nc.dram_tensor("y", [t_tokens, out_s], f32, kind="ExternalOutput").ap()

    # output column chunks, each <= 512 (one PSUM bank)
    o_chunks = []
    o0 = 0
    while o0 < out_s:
        o_chunks.append((o0, min(o0 + 512, out_s)))
        o0 += 512

    with tile.TileContext(nc) as tc:
        with (
            tc.tile_pool(name="consts", bufs=1) as consts,
            tc.tile_pool(name="xp", bufs=2) as xp,
        ):
            # awq reciprocal first: it gates the first dequant op
            awq_sb = consts.tile([GS, G], f32)
            nc.sync.dma_start(awq_sb[:], awq.rearrange("(g p) -> p g", p=GS))
            awq_inv = consts.tile([GS, G], f32)
            nc.vector.reciprocal(awq_inv[:], awq_sb[:])

            # x chunk loads: one DMA per 128-token tile so matmuls gate on
            # small pieces; chunk 0 prefetched during dequant
            def load_x(c):
                t = xp.tile([GS, n_tt, G, 128], bf16, tag="x", name=f"x_{c}")
                for tt in range(n_tt):
                    nc.sync.dma_start(t[:, tt], xt[c, tt])
                return t

            x_tiles = {0: load_x(0)}
            bias_b = consts.tile([GS, out_s], f32)
            # resident dequantized weights [k_in_group, group, out]
            wp = consts.tile([GS, G, out_s], bf16)

            # --- dequantize, one 128-row group at a time ---
            with tc.tile_pool(name="dq", bufs=2 if use_zero else 4) as dq:
                for g in range(G):
                    wint_t = dq.tile([GS, out_s], bf16, tag="wint",
                                     name=f"wint_{g}")
                    nc.sync.dma_start(wint_t[:], wT[g * GS:(g + 1) * GS, :])
                    # broadcast scale row g across partitions via DMA
                    # (AXI side; keeps the DVE/GpSimd SBUF port free)
                    sc_b = dq.tile([GS, out_s], f32, tag="sc", name=f"sc_{g}")
                    nc.sync.dma_start(
                        sc_b[:], scaleT[g:g + 1, :].to_broadcast([GS, out_s]))
                    if use_zero:
                        z_b = dq.tile([GS, out_s], f32, tag="z", name=f"z_{g}")
                        nc.sync.dma_start(
                            z_b[:], zeroT[g:g + 1, :].to_broadcast([GS, out_s]))
                        t0 = dq.tile([GS, out_s], f32, tag="t0", name=f"t0_{g}")
                        nc.vector.tensor_sub(t0[:], wint_t[:], z_b[:])
                    else:
                        t0 = wint_t
                    # wp = (t0 * awq_inv[k]) * scale_bcast
                    nc.vector.scalar_tensor_tensor(
                        wp[:, g, :], t0[:], awq_inv[:, g:g + 1], sc_b[:],
                        mybir.AluOpType.mult, mybir.AluOpType.mult)

            # bias broadcast is only needed by the first tensor_add; its
            # slow 128x-reread DMA must not block the startup critical path
            nc.sync.dma_start(bias_b[:], bias[None, :].to_broadcast([GS, out_s]))

            # --- matmul: out[t,o] = sum_k x[k,t] * wp[k,o] ---
            with (
                tc.tile_pool(name="outp", bufs=3) as outp,
                tc.tile_pool(name="ps", bufs=2 * len(o_chunks),
                             space="PSUM") as ps,
            ):
                def finish_tile(c, tt, out_sb, psts):
                    trow = c * t_chunk + tt * 128
                    for oc, (a, b) in enumerate(o_chunks):
                        nc.any.tensor_add(
                            out_sb[:, a:b], psts[oc][:, :b - a], bias_b[:, a:b])
                    nc.sync.dma_start(y[trow:trow + 128, :], out_sb[:])

                for c in range(n_ch):
                    if c not in x_tiles:
                        x_tiles[c] = load_x(c)
                    x_sb = x_tiles[c]
                    for tt in range(n_tt):
                        out_sb = outp.tile([128, out_s], f32, tag="out",
                                           name=f"out_{c}_{tt}")
                        psts = []
                        # PSUM-bank-stable order: all K accumulations for one
                        # output chunk back-to-back into a single bank
                        for oc, (a, b) in enumerate(o_chunks):
                            pst = ps.tile([128, 512], f32, tag="ps",
                                          name=f"ps_{c}_{tt}_{oc}")
                            psts.append(pst)
                            for k in range(G):
                                nc.tensor.matmul(
                                    pst[:, :b - a],
                                    x_sb[:, tt, k, :],
                                    wp[:, k, a:b],
                                    start=(k == 0), stop=(k == G - 1))
                        finish_tile(c, tt, out_sb, psts)

    nc.compile()
    return nc


def make_in_maps(x, weight_int, scale_per_group, zero_per_group, awq_scale,
                 bias, out_s=OUT_S, n_cores=N_CORES, use_zero=True,
                 t_chunk=None):
    """Shard + lay out host inputs for the 8 cores."""
    x = np.asarray(x, dtype=np.float32)
    t_tokens = x.shape[0]
    if t_chunk is None:
        t_chunk = chunk_for(use_zero, t_tokens)
    t_chunk = min(t_chunk, t_tokens)
    n_ch = t_tokens // t_chunk
    # [chunk, ttile, k_in_group, group, token128]: contiguous (g, t) rows
    n_tt = t_chunk // 128
    xt = np.ascontiguousarray(
        x.astype(BF16).T                      # (IN, T)
        .reshape(G, GS, n_ch, n_tt, 128)      # (g, p, c, tt, t)
        .transpose(2, 3, 1, 0, 4))            # (c, tt, p, g, t)
    awq_f = np.ascontiguousarray(np.asarray(awq_scale, dtype=np.float32))
    in_maps = []
    for s in range(n_cores):
        sl = slice(s * out_s, (s + 1) * out_s)
        m = {
            "xt": xt,
            # int4 values are exact in bf16
            "wT": np.ascontiguousarray(
                np.asarray(weight_int)[sl].T.astype(BF16)),
            "scaleT": np.ascontiguousarray(
                np.asarray(scale_per_group, dtype=np.float32)[sl].T),
            "awq": awq_f,
            "bias": np.ascontiguousarray(
                np.asarray(bias, dtype=np.float32)[sl]),
        }
        if use_zero:
            m["zeroT"] = np.ascontiguousarray(
                np.asarray(zero_per_group, dtype=np.float32)[sl].T)
        in_maps.append(m)
    return in_maps


_NC_CACHE = {}


def _get_nc(use_zero=True):
    key = (T_FULL, OUT_S, use_zero)
    if key not in _NC_CACHE:
        _NC_CACHE[key] = build_nc(use_zero=use_zero)
    return _NC_CACHE[key]


def kernel(x, weight_int, scale_per_group, zero_per_group, awq_scale, bias,
           **_kw):
    use_zero = bool(np.any(np.asarray(zero_per_group)))
    in_maps = make_in_maps(x, weight_int, scale_per_group, zero_per_group,
                           awq_scale, bias, use_zero=use_zero)
    nc = _get_nc(use_zero=use_zero)
    res = run_bass_kernel_spmd(nc, in_maps, core_ids=list(range(N_CORES)))
    y = np.concatenate([res.results[s]["y"] for s in range(N_CORES)], axis=1)
    return np.ascontiguousarray(y, dtype=np.float32)
